# revision 22
# baseline (speedup 1.0000x reference)
"""Causal attention kernel for Trainium2, SPMD over 8 NeuronCores.

Problem (hardcoded): embeddings [4, 2048, 1024] f32, Wq/Wk/Wv [1024, 1024] f32.
    q = X Wq; k = X Wk; v = X Wv
    out = softmax(causal(q k^T) / 32) v          (per batch)

Sharding: 8 cores = (4 batches) x (2 q-shards). Each core handles 1024 query
rows of one batch, chosen as eight 128-row q-tiles with balanced causal work:
core parity 0 gets the even global q-tiles [0,2,..,14], parity 1 the odd ones.
Both see the same per-slot k-extent pattern [1..8] (in 256-wide k-slices) and
a single causal-mask pattern (offset 0 or 128), so one SPMD program serves
all 8 cores; all per-core divergence is carried by input data.

Host->device traffic is minimized (the per-call input upload dominates the
measured time): each core uploads ONLY its own 1024 q-columns of X_b^T in
fp16 (2 MB) plus a 128-row shard of the weights (0.5 MB). On device, an
AllGather over core pairs (0,1),(2,3),... reunites the two q-column sets of
each batch -- together they are ALL columns of X_b^T, interleaved in a
parity-independent pattern (ordered 128-col tile t lives in gathered half
t%2 at block t//2). An 8-way AllGather reunites the weight shards.

Algebraic restructure (from v0): S = Q K^T = Xq (Wq Wk^T) X^T. The host
precomputes wm = Wq @ Wk.T once; on-device G^T = wm^T @ Xq^T (one 1024-row
projection instead of Q and a 2048-row K), then S = G X^T. V = X Wv from
the same resident X^T. P = exp(S/32 + mask) unnormalized (logits are O(6),
exp is safe); O = (P V) * 1/rowsum(P). All matmuls run in fp16 (full PE
rate); accumulation is f32 in PSUM. Output returns as fp16 (host upcasts).
"""

import numpy as np

B = 4
S = 2048
E = 1024
D = 1024
P = 128
NCORES = 8
KSL = 512  # matmul moving-dim chunk
KA = 256  # attention k-slice width
EO = E // P  # 8 e-chunks
KT = S // P  # 16 k-tiles
NQ = P * 8  # 1024 q rows per core

# global q-tile indices per core parity: even tiles vs odd tiles. Both give
# the per-slot k-extent pattern [1..8] in 256-wide k-slices, and a single
# 128-row causal mask pattern per core (offset 0 or 128).
TILES = [
    [0, 2, 4, 6, 8, 10, 12, 14],
    [1, 3, 5, 7, 9, 11, 13, 15],
]
CNT = [1, 2, 3, 4, 5, 6, 7, 8]  # 256-wide k-slices per slot (t // 2 + 1)

MASK_VAL = -1.0e30

_CACHE = {}


def _build_program():
    import concourse.bacc as bacc
    import concourse.tile as tile
    from concourse import mybir
    f16 = mybir.dt.float16
    bf16 = mybir.dt.bfloat16
    f32 = mybir.dt.float32

    nc = bacc.Bacc("TRN2", target_bir_lowering=False, debug=False, num_devices=NCORES)

    xqt_d = nc.dram_tensor("xqt", [E, NQ], bf16, kind="ExternalInput")  # my q-cols
    ident_d = nc.dram_tensor("ident", [P, P], bf16, kind="ExternalInput")
    wsh_d = nc.dram_tensor("wsh", [2, P, E], bf16, kind="ExternalInput")  # wm/wv rows
    mask_d = nc.dram_tensor("masks", [P, KA], bf16, kind="ExternalInput")
    out_d = nc.dram_tensor("out", [8, P, D], f16, kind="ExternalOutput")

    with tile.TileContext(nc) as tc:
        with (
            tc.tile_pool(name="dram", bufs=1, space="DRAM") as dram,
            tc.tile_pool(name="persist", bufs=1) as persist,
            tc.tile_pool(name="psS", bufs=3, space="PSUM") as psS,
            tc.tile_pool(name="psT", bufs=3, space="PSUM") as psT,
            tc.tile_pool(name="psO", bufs=2, space="PSUM") as psO,
        ):
            # ---- collectives: reunite X columns (pairs) and weights (all) ----
            # Order matters: wm first (gt depends only on it), then X pair
            # gather (V/S), then wv (V). gt overlaps the later gathers.
            xq_b = dram.tile([E, NQ], bf16)
            xg = dram.tile([2, EO, P, NQ], bf16)
            w_b = dram.tile([2, P, E], bf16)
            wg = dram.tile([EO, 2, P, E], bf16, addr_space="Shared")

            nc.sync.dma_start(w_b[:], wsh_d[:])
            nc.scalar.dma_start(xq_b[: E // 2], xqt_d[: E // 2])
            nc.sync.dma_start(xq_b[E // 2 :], xqt_d[E // 2 :])
            nc.gpsimd.collective_compute(
                "AllGather",
                mybir.AluOpType.bypass,
                replica_groups=[list(range(NCORES))],
                ins=[w_b[:].opt()],
                outs=[wg[:].opt()],
            )
            # scheduler fence: keep the X pair-gather strictly after the
            # weight gather on the cc stream (the scheduler otherwise
            # reorders them, and gt -- which only needs wm -- then stalls
            # behind every collective)
            tc.no_sync_barrier()
            nc.gpsimd.collective_compute(
                "AllGather",
                mybir.AluOpType.bypass,
                replica_groups=[[2 * i, 2 * i + 1] for i in range(NCORES // 2)],
                ins=[xq_b[:].opt()],
                outs=[xg[:].opt()],
            )

            # ---- persistent SBUF ----
            # identity comes from the host: make_identity's gpsimd
            # affine_select sits in front of the kernel-barrier doorbell on
            # the gpsimd queue and delays every collective by ~14us
            ident = persist.tile([P, P], bf16, tag="ident")
            nc.sync.dma_start(ident, ident_d[:])
            masks_b = persist.tile([P, KA], bf16, tag="masks_b")
            nc.sync.dma_start(masks_b, mask_d[:])
            masks_sb = persist.tile([P, KA], f32, tag="masks")
            nc.vector.tensor_copy(masks_sb, masks_b)

            xqt_sb = persist.tile([P, EO, NQ], bf16, tag="xqt")  # Xq^T [e, q]
            wm_sb = persist.tile([P, EO, E], bf16, tag="wm")
            wv_sb = persist.tile([P, EO, D], bf16, tag="wv")
            gt = persist.tile([P, EO, NQ], bf16, tag="gt")  # G^T [e, q]
            xt = persist.tile([P, EO, S], bf16, tag="xt")  # X^T [e, s] ordered
            v = persist.tile([P, KT, D], bf16, tag="v")  # V [k, dv]

            # my q-cols straight from the input (no collective dependency)
            xqt_r = xqt_d.rearrange("(eo ei) q -> ei eo q", ei=P)
            for eo in range(EO):
                nc.sync.dma_start(xqt_sb[:, eo, :], xqt_r[:, eo, :])

            # weights from the 8-way gathers: rank co contributed rows
            # [128co, 128(co+1)). On sync/scalar queues (sem-gated by the
            # collective), NOT gpsimd (whose queue serializes behind later
            # collectives).
            for co in range(EO):
                nc.sync.dma_start(wm_sb[:, co, :], wg[co, 0])
            for eo in range(EO):
                nc.sync.dma_start(wv_sb[:, eo, :], wg[eo, 1])

            # ordered X^T from the pair gather: ordered tile t = half t%2,
            # block t//2. One DMA per (half, eo).
            xt5 = xt.rearrange("ei eo (k h i) -> ei eo h k i", h=2, i=P)
            for h in range(2):
                for eo in range(EO):
                    src = xg[h, eo].rearrange("p (k i) -> p k i", i=P)
                    nc.scalar.dma_start(xt5[:, eo, h], src)

            # ---- G^T = wm^T @ Xq^T ----
            for qh in range(2):
                for et in range(EO):
                    ps = psS.tile([P, KSL], f32, tag="ps", name="ps_gt")
                    for co in range(EO):
                        nc.tensor.matmul(
                            ps,
                            wm_sb[:, co, et * P : (et + 1) * P],
                            xqt_sb[:, co, qh * KSL : (qh + 1) * KSL],
                            start=(co == 0),
                            stop=(co == EO - 1),
                        )
                    nc.vector.tensor_copy(gt[:, et, qh * KSL : (qh + 1) * KSL], ps)

            # ---- V = X @ wv ----
            for kt in range(KT):
                for dvh in range(2):
                    ps = psS.tile([P, KSL], f32, tag="ps", name="ps_v")
                    for eo in range(EO):
                        nc.tensor.matmul(
                            ps,
                            xt[:, eo, kt * P : (kt + 1) * P],
                            wv_sb[:, eo, dvh * KSL : (dvh + 1) * KSL],
                            start=(eo == 0),
                            stop=(eo == EO - 1),
                        )
                    nc.vector.tensor_copy(v[:, kt, dvh * KSL : (dvh + 1) * KSL], ps)

            # ---- attention: S, exp, transposes, then PV ----
            with tc.tile_pool(name="attn", bufs=1) as attn:
                pt = attn.tile([P, 8, 16, P], bf16, tag="pt", bufs=1)
                stats = attn.tile([P, 8, 12], f32, tag="stats", bufs=1)
                for s_slot in range(8):
                    c = CNT[s_slot]
                    # S in 512-wide slabs (adjacent 256-slice pairs fused),
                    # plus a 256 tail when c is odd. The causal mask lands on
                    # the last 256 columns.
                    slabs = [(si * 2, 512) for si in range(c // 2)]
                    if c % 2:
                        slabs.append((c - 1, 256))
                    nslab = len(slabs)
                    for si, (j0, width) in enumerate(slabs):
                        ps = psS.tile([P, KSL], f32, tag="ps", name="ps_s")[:, :width]
                        for eo in range(EO):
                            nc.tensor.matmul(
                                ps,
                                gt[:, eo, s_slot * P : (s_slot + 1) * P],
                                xt[:, eo, j0 * KA : j0 * KA + width],
                                start=(eo == 0),
                                stop=(eo == EO - 1),
                            )
                        if si == nslab - 1:
                            nc.vector.tensor_add(
                                ps[:, width - KA :], ps[:, width - KA :], masks_sb
                            )
                        p_sb = attn.tile([P, KSL], bf16, tag="p", bufs=3, name="p_sb")[
                            :, :width
                        ]
                        nc.scalar.activation(
                            p_sb,
                            ps,
                            mybir.ActivationFunctionType.Exp,
                            bias=0.0,
                            scale=1.0 / 32.0,
                            accum_out=stats[:, s_slot, si : si + 1],
                        )
                        for t4 in range(width // P):
                            pst = psT.tile([P, P], bf16)
                            nc.tensor.transpose(
                                pst, p_sb[:, t4 * P : (t4 + 1) * P], ident
                            )
                            nc.vector.tensor_copy(pt[:, s_slot, 2 * j0 + t4, :], pst)

                    # l = sum_si stats[:, si]; r = 1 / l
                    nc.vector.reduce_sum(
                        stats[:, s_slot, 8:9],
                        stats[:, s_slot, 0:nslab],
                        axis=mybir.AxisListType.X,
                    )
                    nc.vector.reciprocal(stats[:, s_slot, 9:10], stats[:, s_slot, 8:9])

                # ---- attention phase B: PV (after the V pair-gather) ----
                for s_slot in range(8):
                    c = CNT[s_slot]
                    out_r = out_d[s_slot].rearrange("p (h k) -> p h k", h=2)
                    for dvh in range(2):
                        pso = psO.tile([P, KSL], f32, tag="o", name=f"pso_{dvh}")
                        for kt in range(2 * c):
                            nc.tensor.matmul(
                                pso,
                                pt[:, s_slot, kt, :],
                                v[:, kt, dvh * KSL : (dvh + 1) * KSL],
                                start=(kt == 0),
                                stop=(kt == 2 * c - 1),
                            )
                        o_sb = attn.tile([P, KSL], f16, tag="o", bufs=2, name="o_sb")
                        nc.vector.tensor_scalar_mul(o_sb, pso, stats[:, s_slot, 9:10])
                        nc.sync.dma_start(out_r[:, dvh, :], o_sb)

    nc.compile()
    return nc


def _get_program():
    key = "nc"
    if key not in _CACHE:
        _CACHE[key] = _build_program()
    return _CACHE[key]


def _host_masks(parity):
    """mask[r, col]: 0 where col <= 128*parity + r else MASK_VAL."""
    col = np.arange(KA)[None, :]
    row = np.arange(P)[:, None]
    import ml_dtypes

    return np.where(col <= 128 * parity + row, 0.0, MASK_VAL).astype(
        ml_dtypes.bfloat16
    )


def _in_maps(embeddings, Wq, Wk, Wv):
    import ml_dtypes

    bf16 = ml_dtypes.bfloat16
    wm = (Wq @ Wk.T).astype(bf16)
    wvh = Wv.astype(bf16)
    maps = []
    for c in range(NCORES):
        b, g = divmod(c, 2)
        Xb = embeddings[b]
        xq = np.concatenate([Xb[P * t : P * (t + 1)] for t in TILES[g]], axis=0)
        xqt = np.ascontiguousarray(xq.T.astype(bf16))
        wsh = np.ascontiguousarray(
            np.stack([wm[P * c : P * (c + 1)], wvh[P * c : P * (c + 1)]])
        )
        maps.append(
            {
                "xqt": xqt,
                "wsh": wsh,
                "masks": _host_masks(g),
                "ident": np.eye(P, dtype=bf16),
            }
        )
    return maps


def _run(embeddings, Wq, Wk, Wv, **spmd_kwargs):
    from concourse.bass_utils import run_bass_kernel_spmd

    nc = _get_program()
    maps = _in_maps(embeddings, Wq, Wk, Wv)
    res = run_bass_kernel_spmd(nc, maps, core_ids=list(range(NCORES)), **spmd_kwargs)
    out = np.empty((B, S, D), np.float32)
    for c in range(NCORES):
        b, g = divmod(c, 2)
        oc = np.asarray(res.results[c]["out"]).astype(np.float32)
        for s_slot, t in enumerate(TILES[g]):
            out[b, P * t : P * (t + 1), :] = oc[s_slot]
    return out, res


def kernel(embeddings, Wq, Wk, Wv):
    embeddings = np.ascontiguousarray(np.asarray(embeddings, dtype=np.float32))
    Wq = np.ascontiguousarray(np.asarray(Wq, dtype=np.float32))
    Wk = np.ascontiguousarray(np.asarray(Wk, dtype=np.float32))
    Wv = np.ascontiguousarray(np.asarray(Wv, dtype=np.float32))
    out, _ = _run(embeddings, Wq, Wk, Wv)
    return out
